# revision 1
# baseline (speedup 1.0000x reference)
"""GNN message-passing (EGNN-style classifier) on 8 TRN2 NeuronCores.

Data-parallel over ligands: each core handles 128 ligands = 4096 nodes,
32768 edges (edges never cross ligands). Weights replicated.

Device layout (per core):
- Node state hh kept feature-major [128 feats, 4096 nodes] in SBUF (f32 master
  + bf16 copy for matmul inputs).
- Edge pipeline per layer, per group of 1024 edges (8 chunks x 128 edges):
  m1_pre = hh[row] @ A + hh[col] @ B + edge_attr @ C computed edge-major via
  three PE matmuls per chunk (R-gather / one-hot gather / edge-attr lhsT).
  LayerNorm stats via DVE bn_stats on PSUM; fused scale/bias+SiLU on ACT
  (edge-major -> per-partition scalars). DMA-transpose to feature-major,
  We2 matmul, SiLU, attention via PE (Watt column / mij_fm lhsT), gated
  segment-sum via one-hot-weighted (S*att) matmuls back to node-major.
- Node MLP node-major with the same LN trick; residual update in f32.
"""
import numpy as np
import ml_dtypes

N_LIG = 1024
K = 32                 # atoms per ligand
N = N_LIG * K          # 32768 nodes
KNN = 8
E = N * KNN            # 262144 edges
IN_F = 16
T_F = 16
HID = 128
OUT_F = 64
DEPTH = 4
NG = 20
NT = 1000
EDGE_IN = NG + T_F
NORM_FACTOR = 5.0
EPS = 1e-5

NCORES = 8
NLc = N // NCORES      # 4096 nodes / core
NEc = E // NCORES      # 32768 edges / core
LIGc = N_LIG // NCORES  # 128 ligands / core
NCHUNK = NEc // 128    # 256 edge chunks / core
NGRP = NCHUNK // 8     # 32 groups of 1024 edges

bf16 = ml_dtypes.bfloat16

# Gaussian smearing constants
_off = np.exp(np.linspace(np.log(1.0), np.log(5.0), NG)) - 1.0
_d = np.diff(_off)
_d = np.concatenate([_d[:1], _d])
GS_OFFSET = _off.astype(np.float32)
GS_COEFF = (-0.5 / _d ** 2).astype(np.float32)

_COMPILED = {}


def _build_program():
    import concourse.bacc as bacc
    import concourse.bass as bass
    import concourse.mybir as mybir
    import concourse.tile as tile

    bf = mybir.dt.bfloat16
    f32 = mybir.dt.float32
    AF = mybir.ActivationFunctionType
    ALU = mybir.AluOpType

    nc = bacc.Bacc("TRN2", target_bir_lowering=False, debug=False)

    # ---------------- DRAM tensors ----------------
    d_in_fm = nc.dram_tensor("in_fm", [32, NLc], bf, kind="ExternalInput")
    d_ea = nc.dram_tensor("ea", [128, 16384], bf, kind="ExternalInput")
    d_onehot = nc.dram_tensor("onehot", [128, 8192], bf, kind="ExternalInput")
    d_R = nc.dram_tensor("Rall", [128, 256], bf, kind="ExternalInput")
    d_S = nc.dram_tensor("Spat", [128, 256], bf, kind="ExternalInput")
    # per-layer weights (stacked on the free axis)
    d_Aaug = nc.dram_tensor("Aaug", [128, DEPTH, 129], bf, kind="ExternalInput")
    d_Baug = nc.dram_tensor("Baug", [128, DEPTH, 129], bf, kind="ExternalInput")
    d_Caug = nc.dram_tensor("Caug", [128, DEPTH, 129], bf, kind="ExternalInput")  # rows 0:36 and 64:100
    d_We2 = nc.dram_tensor("We2", [128, DEPTH, 128], bf, kind="ExternalInput")
    d_Watt = nc.dram_tensor("Watt", [128, DEPTH, 1], bf, kind="ExternalInput")
    d_N1 = nc.dram_tensor("N1aug", [128, DEPTH, 2, 129], bf, kind="ExternalInput")  # [.,l,0,:]=hh-part, [.,l,1,:]=agg-part
    d_Wn2 = nc.dram_tensor("Wn2", [128, DEPTH, 128], bf, kind="ExternalInput")
    d_Win = nc.dram_tensor("Win", [32, 128], bf, kind="ExternalInput")
    d_Woe = nc.dram_tensor("Woe", [128, 64], bf, kind="ExternalInput")
    d_pool = nc.dram_tensor("poolpat", [128, 4], bf, kind="ExternalInput")
    d_Wf = nc.dram_tensor("Wf", [64, 1], f32, kind="ExternalInput")
    d_out = nc.dram_tensor("out", [1, LIGc], f32, kind="ExternalOutput")

    with tile.TileContext(nc) as tc:
        with tc.tile_pool(name="stat", bufs=1) as stat, \
             tc.tile_pool(name="hhp", bufs=1) as hhp, \
             tc.tile_pool(name="stg", bufs=4) as stg, \
             tc.tile_pool(name="sml", bufs=6) as sml, \
             tc.tile_pool(name="ps1", bufs=2, space="PSUM") as ps1, \
             tc.tile_pool(name="ps2", bufs=1, space="PSUM") as ps2, \
             tc.tile_pool(name="ps3", bufs=2, space="PSUM") as ps3:

            # ---------- static loads ----------
            t_ea = stat.tile([128, 16384], bf, tag="t_ea")
            nc.sync.dma_start(t_ea[:], d_ea[:])
            t_oh = stat.tile([128, 8192], bf, tag="t_oh")
            nc.sync.dma_start(t_oh[:], d_onehot[:])
            t_R = stat.tile([128, 256], bf, tag="t_R")
            nc.sync.dma_start(t_R[:], d_R[:])
            t_S = stat.tile([128, 256], bf, tag="t_S")
            nc.sync.dma_start(t_S[:], d_S[:])
            t_Aaug = stat.tile([128, DEPTH, 129], bf, tag="t_Aaug")
            nc.sync.dma_start(t_Aaug[:], d_Aaug[:])
            t_Baug = stat.tile([128, DEPTH, 129], bf, tag="t_Baug")
            nc.sync.dma_start(t_Baug[:], d_Baug[:])
            t_Caug = stat.tile([128, DEPTH, 129], bf, tag="t_Caug")
            nc.sync.dma_start(t_Caug[:], d_Caug[:])
            t_We2 = stat.tile([128, DEPTH, 128], bf, tag="t_We2")
            nc.sync.dma_start(t_We2[:], d_We2[:])
            t_Watt = stat.tile([128, DEPTH, 1], bf, tag="t_Watt")
            nc.sync.dma_start(t_Watt[:], d_Watt[:])
            t_N1 = stat.tile([128, DEPTH, 2, 129], bf, tag="t_N1")
            nc.sync.dma_start(t_N1[:], d_N1[:])
            t_Wn2 = stat.tile([128, DEPTH, 128], bf, tag="t_Wn2")
            nc.sync.dma_start(t_Wn2[:], d_Wn2[:])
            t_Win = stat.tile([32, 128], bf, tag="t_Win")
            nc.sync.dma_start(t_Win[:], d_Win[:])
            t_Woe = stat.tile([128, 64], bf, tag="t_Woe")
            nc.sync.dma_start(t_Woe[:], d_Woe[:])
            t_pool = stat.tile([128, 4], bf, tag="t_pool")
            nc.sync.dma_start(t_pool[:], d_pool[:])
            t_Wf = stat.tile([64, 1], f32, tag="t_Wf")
            nc.sync.dma_start(t_Wf[:], d_Wf[:])
            t_in = stat.tile([32, NLc], bf, tag="t_in")
            nc.sync.dma_start(t_in[:], d_in_fm[:])
            t_eps = stat.tile([128, 1], f32, tag="t_eps")
            nc.vector.memset(t_eps[:], EPS)

            # ---------- persistent node state ----------
            hh_f = hhp.tile([128, NLc], f32, tag="hh_f")
            hh_b = hhp.tile([128, NLc], bf, tag="hh_b")
            agg_fm = hhp.tile([128, NLc], bf, tag="agg_fm")
            nm_fm = hhp.tile([128, NLc], bf, tag="nm_fm")
            nodeA = hhp.tile([128, 32, 129], bf, tag="nodeA")
            nodeB = hhp.tile([128, 32, 129], bf, tag="nodeB")
            att_em = hhp.tile([128, NCHUNK], f32, tag="att_em")

            # ---------- prologue: hh0 = [h|emb] @ Win ----------
            for nb in range(8):
                p = ps2.tile([128, 2, 512], f32, tag="v2")
                nc.tensor.matmul(p[:, 0, :], lhsT=t_Win[:], rhs=t_in[:, 512 * nb:512 * nb + 512],
                                 start=True, stop=True)
                nc.scalar.activation(hh_f[:, 512 * nb:512 * nb + 512], p[:, 0, :],
                                     AF.Copy)
                nc.vector.tensor_copy(hh_b[:, 512 * nb:512 * nb + 512], p[:, 0, :])

            # ---------- layers ----------
            for l in range(DEPTH):
                # nodeA/nodeB (node-major, 129 cols incl aug-mean)
                for nb in range(32):
                    pn = ps1.tile([128, 2, 512], f32, tag="m1pre")
                    nc.tensor.matmul(pn[:, 0, 0:129], lhsT=hh_b[:, 128 * nb:128 * nb + 128],
                                     rhs=t_Aaug[:, l, :], start=True, stop=True)
                    nc.tensor.matmul(pn[:, 1, 0:129], lhsT=hh_b[:, 128 * nb:128 * nb + 128],
                                     rhs=t_Baug[:, l, :], start=True, stop=True)
                    nc.scalar.activation(nodeA[:, nb, :], pn[:, 0, 0:129], AF.Copy)
                    nc.vector.tensor_copy(nodeB[:, nb, :], pn[:, 1, 0:129])

                for g in range(NGRP):
                    # ---- m1_pre: process in 2 halves of 4 chunks (2 psum tiles) ----
                    m1_em = stg.tile([128, 1024], bf, tag="m1_em")
                    m1_fm = stg.tile([128, 1024], bf, tag="m1_fm")
                    for half in range(2):
                        pts = []
                        for hh2 in range(2):
                            pt = ps1.tile([128, 2, 512], f32, tag="m1pre")
                            pts.append(pt)
                        mv4 = sml.tile([128, 4, 2], f32, tag="mv4")
                        st4 = sml.tile([128, 4, 6], f32, tag="st4")
                        for jj in range(4):
                            j = 4 * half + jj
                            c = 8 * g + j
                            L = c // 2
                            base = 32 * (L % 4)
                            hs = c % 2
                            eh = 0 if c < 128 else 1
                            pt = pts[jj // 2]
                            sl = pt[:, jj % 2, 0:129]
                            nc.tensor.matmul(sl, lhsT=t_R[base:base + 32, 128 * hs:128 * hs + 128],
                                             rhs=nodeA[base:base + 32, L // 4, :],
                                             start=True, stop=False, tile_position=(base, 0))
                            ohf = 128 * (2 * (c // 8) + hs)
                            nc.tensor.matmul(sl, lhsT=t_oh[base:base + 32, ohf:ohf + 128],
                                             rhs=nodeB[base:base + 32, L // 4, :],
                                             start=False, stop=False, tile_position=(base, 0))
                            nc.tensor.matmul(sl, lhsT=t_ea[64 * eh:64 * eh + 36, 128 * (c % 128):128 * (c % 128) + 128],
                                             rhs=t_Caug[64 * eh:64 * eh + 36, l, :],
                                             start=False, stop=True, tile_position=(64 * eh, 0))
                            nc.vector.bn_stats(st4[:, jj, :], pt[:, jj % 2, 0:128])
                            nc.vector.bn_aggr(mv4[:, jj, :], st4[:, jj, :])
                        rstd4 = sml.tile([128, 4], f32, tag="rstd4")
                        nmr4 = sml.tile([128, 4], f32, tag="nmr4")
                        nc.scalar.activation(rstd4[:], mv4[:, :, 1], AF.Sqrt, bias=t_eps[:], scale=1.0)
                        nc.vector.reciprocal(rstd4[:], rstd4[:])
                        nc.vector.scalar_tensor_tensor(nmr4[:], in0=mv4[:, :, 0], scalar=-1.0,
                                                       in1=rstd4[:], op0=ALU.mult, op1=ALU.mult)
                        for jj in range(4):
                            j = 4 * half + jj
                            pt = pts[jj // 2]
                            nc.scalar.activation(m1_em[:, 128 * j:128 * j + 128], pt[:, jj % 2, 0:128],
                                                 AF.Silu, bias=nmr4[:, jj:jj + 1], scale=rstd4[:, jj:jj + 1])
                            nc.sync.dma_start_transpose(m1_fm[:, 128 * j:128 * j + 128],
                                                        m1_em[:, 128 * j:128 * j + 128])
                    # We2 -> v2 (feature-major) + SiLU -> mij_fm bf16
                    pv2 = ps2.tile([128, 2, 512], f32, tag="v2")
                    nc.tensor.matmul(pv2[:, 0, :], lhsT=t_We2[:, l, :], rhs=m1_fm[:, 0:512],
                                     start=True, stop=True)
                    nc.tensor.matmul(pv2[:, 1, :], lhsT=t_We2[:, l, :], rhs=m1_fm[:, 512:1024],
                                     start=True, stop=True)
                    mij_fm = stg.tile([128, 1024], bf, tag="mij_fm")
                    nc.scalar.activation(mij_fm[:], pv2[:].rearrange("p a b -> p (a b)"), AF.Silu)
                    # att: edge-major [128,1] per chunk via mij_fm as lhsT
                    patt = ps3.tile([128, 512], f32, tag="aggatt")
                    for j in range(8):
                        nc.tensor.matmul(patt[:, j:j + 1], lhsT=mij_fm[:, 128 * j:128 * j + 128],
                                         rhs=t_Watt[:, l, :], start=True, stop=True)
                    nc.scalar.activation(att_em[:, 8 * g:8 * g + 8], patt[:, 0:8], AF.Sigmoid)
                    # S*att (bf16) via bcast-TT
                    satt = stg.tile([128, 256], bf, tag="satt")
                    att_bc = bass.AP(tensor=att_em[:].tensor, offset=att_em[:, 8 * g:8 * g + 8].offset,
                                     ap=[att_em[:].ap[0], [1, 8], [0, 32]])
                    nc.vector.tensor_tensor(out=satt[:].rearrange("p (a b) -> p a b", a=8),
                                            in0=t_S[:].rearrange("p (a b) -> p a b", a=8),
                                            in1=att_bc, op=ALU.mult)
                    # mij back to edge-major
                    mij_em = stg.tile([128, 1024], bf, tag="mij_em")
                    for j in range(8):
                        nc.sync.dma_start_transpose(mij_em[:, 128 * j:128 * j + 128],
                                                    mij_fm[:, 128 * j:128 * j + 128])
                    # gated segment-sum -> node-major agg [128 nodes, 128]
                    pagg = ps3.tile([128, 512], f32, tag="aggatt")
                    for j in range(8):
                        nc.tensor.matmul(pagg[32 * (j // 2):32 * (j // 2) + 32, 0:128],
                                         lhsT=satt[:, 32 * j:32 * j + 32],
                                         rhs=mij_em[:, 128 * j:128 * j + 128],
                                         start=(j % 2 == 0), stop=(j % 2 == 1),
                                         tile_position=(0, 32 * (j // 2)))
                    # evac agg (node-major bf16) then transpose to feature-major
                    agg_nm = stg.tile([128, 128], bf, tag="agg_nm")
                    nc.scalar.activation(agg_nm[:], pagg[:, 0:128], AF.Copy)
                    nc.sync.dma_start_transpose(agg_fm[:, 128 * g:128 * g + 128], agg_nm[:])

                # ---- node MLP ----
                for nb in range(16):
                    pn = ps1.tile([128, 2, 512], f32, tag="m1pre")
                    mv2 = sml.tile([128, 2, 2], f32, tag="mv2")
                    st2 = sml.tile([128, 2, 6], f32, tag="st2")
                    for s in range(2):
                        cb = 2 * nb + s
                        sl = pn[:, s, 0:129]
                        nc.tensor.matmul(sl, lhsT=hh_b[:, 128 * cb:128 * cb + 128],
                                         rhs=t_N1[:, l, 0, :], start=True, stop=False)
                        nc.tensor.matmul(sl, lhsT=agg_fm[:, 128 * cb:128 * cb + 128],
                                         rhs=t_N1[:, l, 1, :], start=False, stop=True)
                        nc.vector.bn_stats(st2[:, s, :], pn[:, s, 0:128])
                        nc.vector.bn_aggr(mv2[:, s, :], st2[:, s, :])
                    rstd2 = sml.tile([128, 2], f32, tag="rstd2")
                    nmr2 = sml.tile([128, 2], f32, tag="nmr2")
                    nc.scalar.activation(rstd2[:], mv2[:, :, 1], AF.Sqrt, bias=t_eps[:], scale=1.0)
                    nc.vector.reciprocal(rstd2[:], rstd2[:])
                    nc.vector.scalar_tensor_tensor(nmr2[:], in0=mv2[:, :, 0], scalar=-1.0,
                                                   in1=rstd2[:], op0=ALU.mult, op1=ALU.mult)
                    nm_nm = stg.tile([128, 256], bf, tag="nm_nm")
                    for s in range(2):
                        cb = 2 * nb + s
                        nc.scalar.activation(nm_nm[:, 128 * s:128 * s + 128], pn[:, s, 0:128],
                                             AF.Silu, bias=nmr2[:, s:s + 1], scale=rstd2[:, s:s + 1])
                        nc.sync.dma_start_transpose(nm_fm[:, 128 * cb:128 * cb + 128],
                                                    nm_nm[:, 128 * s:128 * s + 128])
                # hh update: hh += nm @ Wn2
                for nb in range(8):
                    pu = ps2.tile([128, 2, 512], f32, tag="v2")
                    nc.tensor.matmul(pu[:, 0, :], lhsT=t_Wn2[:, l, :],
                                     rhs=nm_fm[:, 512 * nb:512 * nb + 512], start=True, stop=True)
                    nc.vector.tensor_add(hh_f[:, 512 * nb:512 * nb + 512],
                                         hh_f[:, 512 * nb:512 * nb + 512], pu[:, 0, :])
                    nc.vector.tensor_copy(hh_b[:, 512 * nb:512 * nb + 512],
                                          hh_f[:, 512 * nb:512 * nb + 512])

            # ---------- epilogue: ho = hh @ Woe, ligand mean-pool, @ Wf ----------
            pooled_ps = ps3.tile([128, 512], f32, tag="aggatt")
            for nb in range(32):
                ph = ps1.tile([128, 2, 512], f32, tag="m1pre")
                nc.tensor.matmul(ph[:, 0, 0:64], lhsT=hh_b[:, 128 * nb:128 * nb + 128],
                                 rhs=t_Woe[:], start=True, stop=True)
                ho_nm = stg.tile([128, 64], bf, tag="ho_nm")
                nc.scalar.activation(ho_nm[:], ph[:, 0, 0:64], AF.Copy)
                nc.tensor.matmul(pooled_ps[0:64, 4 * nb:4 * nb + 4], lhsT=ho_nm[:],
                                 rhs=t_pool[:], start=True, stop=True)
            pooled_sb = stat.tile([64, 128], f32, tag="pooled_sb")
            nc.vector.tensor_copy(pooled_sb[:], pooled_ps[0:64, 0:128])
            pfin = ps3.tile([128, 512], f32, tag="aggatt")
            nc.tensor.matmul(pfin[0:1, 0:128], lhsT=t_Wf[:], rhs=pooled_sb[:],
                             start=True, stop=True)
            out_sb = stat.tile([1, 128], f32, tag="out_sb")
            nc.vector.tensor_copy(out_sb[:], pfin[0:1, 0:128])
            nc.sync.dma_start(d_out[:], out_sb[:])

    nc.compile()
    return nc


def _prep_inputs(x, h, t, edges, t_bond, batch_ligand, time_emb_table,
                 W_in, gcl_We1, gcl_Wn1, gcl_We2, gcl_Watt, gcl_Wn2,
                 W_oe, W_f):
    """Host-side sharding + static data packing. Returns list of in_maps."""
    row = np.asarray(edges[0])
    col = np.asarray(edges[1])
    assert np.array_equal(row, np.repeat(np.arange(N), KNN)), "row structure"
    assert np.array_equal(np.asarray(batch_ligand), np.arange(N) // K), "batch structure"
    assert np.all(col // K == row // K), "edges cross ligands"

    # edge time-bond embedding (host index arithmetic + table lookups)
    sbi = row * (K - 1) + col - (row // K) * K - (row < col).astype(row.dtype)
    t_bond_e = np.asarray(t_bond)[sbi]
    emb_e = np.asarray(time_emb_table)[t_bond_e]          # [E,16]
    # gaussian smearing (host)
    xx = np.asarray(x)
    cdiff = xx[row] - xx[col]
    radial = (cdiff ** 2).sum(1)
    dist = np.clip(np.sqrt(radial), 0.0, 4.0)
    dd = dist[:, None] - GS_OFFSET[None, :]
    smear = np.exp(GS_COEFF[None, :] * dd * dd)           # [E,20]
    ea = np.concatenate([emb_e, smear], 1).astype(np.float32)  # [E,36]

    emb_t = np.asarray(time_emb_table)[np.asarray(t)]     # [N,16]
    hin = np.concatenate([np.asarray(h), emb_t], 1)       # [N,32]

    # static gather matrices
    col_loc = (col % K).astype(np.int64)                  # atom within ligand

    Rall = np.zeros((128, 256), np.float32)
    for b in range(4):
        for hs in range(2):
            for e in range(128):
                Rall[32 * b + 16 * hs + e // 8, 128 * hs + e] = 1.0
    Spat = np.zeros((128, 256), np.float32)
    for j in range(8):
        for p in range(128):
            Spat[p, 32 * j + 16 * (j % 2) + p // 8] = 1.0

    def aug(W):  # [K,128] -> [K,129] with col 128 = row-wise mean over outputs
        return np.concatenate([W, W.mean(1, keepdims=True)], 1)

    We1 = np.asarray(gcl_We1)  # [D, 292, 128]
    Wn1 = np.asarray(gcl_Wn1)  # [D, 256, 128]
    Aaug = np.zeros((128, DEPTH, 129), np.float32)
    Baug = np.zeros((128, DEPTH, 129), np.float32)
    Caug = np.zeros((128, DEPTH, 129), np.float32)
    N1aug = np.zeros((128, DEPTH, 2, 129), np.float32)
    We2s = np.zeros((128, DEPTH, 128), np.float32)
    Watts = np.zeros((128, DEPTH, 1), np.float32)
    Wn2s = np.zeros((128, DEPTH, 128), np.float32)
    for l in range(DEPTH):
        Aaug[:, l, :] = aug(We1[l][0:128])
        Baug[:, l, :] = aug(We1[l][128:256])
        C = aug(We1[l][256:292])                      # [36,129]
        Caug[0:36, l, :] = C
        Caug[64:100, l, :] = C
        N1aug[:, l, 0, :] = aug(Wn1[l][0:128])
        N1aug[:, l, 1, :] = aug(Wn1[l][128:256] / NORM_FACTOR)
        We2s[:, l, :] = np.asarray(gcl_We2)[l]
        Watts[:, l, :] = np.asarray(gcl_Watt)[l]
        Wn2s[:, l, :] = np.asarray(gcl_Wn2)[l]

    poolpat = np.zeros((128, 4), np.float32)
    for n in range(128):
        poolpat[n, n // 32] = 1.0 / 32.0

    maps = []
    for ci in range(NCORES):
        n0 = ci * NLc
        e0 = ci * NEc
        ea_c = ea[e0:e0 + NEc]                         # [32768, 36]
        ea_pack = np.zeros((128, 16384), np.float32)
        ea_pack[0:36, :] = ea_c[0:16384].T
        ea_pack[64:100, :] = ea_c[16384:32768].T
        col_c = col_loc[e0:e0 + NEc]
        oh = np.zeros((128, 8192), np.float32)
        for c in range(NCHUNK):
            L = c // 2
            base = 32 * (L % 4)
            ohf = 128 * (2 * (c // 8) + (c % 2))
            ee = col_c[128 * c:128 * c + 128]
            oh[base + ee, ohf + np.arange(128)] = 1.0
        m = dict(
            in_fm=np.ascontiguousarray(hin[n0:n0 + NLc].T).astype(bf16),
            ea=ea_pack.astype(bf16),
            onehot=oh.astype(bf16),
            Rall=Rall.astype(bf16),
            Spat=Spat.astype(bf16),
            Aaug=Aaug.astype(bf16),
            Baug=Baug.astype(bf16),
            Caug=Caug.astype(bf16),
            We2=We2s.astype(bf16),
            Watt=Watts.astype(bf16),
            N1aug=N1aug.astype(bf16),
            Wn2=Wn2s.astype(bf16),
            Win=np.asarray(W_in).astype(bf16),
            Woe=np.asarray(W_oe).astype(bf16),
            poolpat=poolpat.astype(bf16),
            Wf=np.asarray(W_f).astype(np.float32),
        )
        maps.append(m)
    return maps


def kernel(x, h, t, edges, t_bond, batch_ligand, num_atoms_per_ligand,
           num_ligands, time_emb_table, W_in, b_in, gcl_We1, gcl_be1, gcl_g1,
           gcl_bt1, gcl_We2, gcl_be2, gcl_Watt, gcl_batt, gcl_Wn1, gcl_bn1,
           gcl_g2, gcl_bt2, gcl_Wn2, gcl_bn2, W_oe, b_oe, W_f, b_f):
    from concourse.bass_utils import run_bass_kernel_spmd

    # all biases zero / gains one in this model family; verify then fold away
    for z in (b_in, gcl_be1, gcl_bt1, gcl_be2, gcl_batt, gcl_bn1, gcl_bt2,
              gcl_bn2, b_oe, b_f):
        assert np.abs(np.asarray(z)).max() == 0.0, "nonzero bias unsupported"
    for o in (gcl_g1, gcl_g2):
        assert np.abs(np.asarray(o) - 1.0).max() == 0.0, "non-unit LN gain"
    assert int(num_atoms_per_ligand) == K and int(num_ligands) == N_LIG

    if "prog" not in _COMPILED:
        _COMPILED["prog"] = _build_program()
    nc = _COMPILED["prog"]

    maps = _prep_inputs(x, h, t, edges, t_bond, batch_ligand, time_emb_table,
                        W_in, gcl_We1, gcl_Wn1, gcl_We2, gcl_Watt, gcl_Wn2,
                        W_oe, W_f)
    res = run_bass_kernel_spmd(nc, maps, list(range(NCORES)))
    out = np.concatenate([r["out"][0] for r in res.results])
    return out.astype(np.float32)



# revision 16
# speedup vs baseline: 1.5497x; 1.5497x over previous
"""GNN message-passing (EGNN-style classifier) on 8 TRN2 NeuronCores.

Data-parallel over ligands: each core handles 128 ligands = 4096 nodes,
32768 edges (edges never cross ligands). Weights replicated.

The axon tunnel moves ~25 MB/s, so the per-dispatch host->device payload
dominates wall clock. All weights and static gather patterns are baked into
the NEFF as Const tensors (shipped once at load); the per-dispatch inputs are
only the true dynamic data (~392 KB/core):
  - hfm   [16,4096]  bf16  node features h, feature-major
  - idxs  [16,2304]  i16   time-emb table indices (edge t_bond + node t),
                           wrapped for gpsimd ap_gather
  - colv  [1,32768]  bf16  col atom index within ligand (0..31)
  - dist2 [2,32768]  bf16  edge distance as hi/lo bf16 pair (sums to f32 dist)
Edge attributes (time-emb gather + Gaussian smearing) and the col one-hot
gather matrix are built on device at startup.

Device layout (per core):
- Node state hh kept feature-major [128 feats, 4096 nodes] in SBUF (f32 master
  + bf16 copy for matmul inputs).
- Edge pipeline per layer, per group of 1024 edges (8 chunks x 128 edges):
  m1_pre = hh[row] @ A + hh[col] @ B + edge_attr @ C computed edge-major via
  three PE matmuls per chunk (R-gather / one-hot gather / edge-attr lhsT).
  LayerNorm stats via DVE bn_stats on PSUM; fused scale/bias+SiLU on ACT.
  DMA-transpose to feature-major, We2 matmul, SiLU, attention via PE,
  gated segment-sum via one-hot-weighted (S*att) matmuls back to node-major.
- Node MLP node-major with the same LN trick; residual update in f32.
- t_ea rows {0:16 emb, 16:32 zero, 32:52 smear} for edges 0:16384 and
  {64:80, 80:96, 96:116} for edges 16384:32768; C weight packed to match.
"""
import numpy as np
import ml_dtypes

N_LIG = 1024
K = 32                 # atoms per ligand
N = N_LIG * K          # 32768 nodes
KNN = 8
E = N * KNN            # 262144 edges
IN_F = 16
T_F = 16
HID = 128
OUT_F = 64
DEPTH = 4
NG = 20
NT = 1000
EDGE_IN = NG + T_F
NORM_FACTOR = 5.0
EPS = 1e-5

NCORES = 8
NLc = N // NCORES      # 4096 nodes / core
NEc = E // NCORES      # 32768 edges / core
LIGc = N_LIG // NCORES  # 128 ligands / core
NCHUNK = NEc // 128    # 256 edge chunks / core
NGRP = NCHUNK // 8     # 32 groups of 1024 edges

bf16 = ml_dtypes.bfloat16

# Gaussian smearing constants
_off = np.exp(np.linspace(np.log(1.0), np.log(5.0), NG)) - 1.0
_d = np.diff(_off)
_d = np.concatenate([_d[:1], _d])
GS_OFFSET = _off.astype(np.float32)
GS_COEFF = (-0.5 / _d ** 2).astype(np.float32)

_COMPILED = {}


def _build_program(W_in, gcl_We1, gcl_Wn1, gcl_We2, gcl_Watt, gcl_Wn2,
                   W_oe, W_f, table, dbg=False):
    import concourse.bacc as bacc
    import concourse.bass as bass
    import concourse.mybir as mybir
    import concourse.tile as tile

    bf = mybir.dt.bfloat16
    f32 = mybir.dt.float32
    i16 = mybir.dt.int16
    AF = mybir.ActivationFunctionType
    ALU = mybir.AluOpType

    nc = bacc.Bacc("TRN2", target_bir_lowering=False, debug=False)

    # ---------------- dynamic inputs (per dispatch) ----------------
    d_hfm = nc.dram_tensor("hfm", [16, NLc], bf, kind="ExternalInput")
    d_idx = nc.dram_tensor("idxs", [16, 2304], i16, kind="ExternalInput")
    d_colv = nc.dram_tensor("colv", [1, NEc], bf, kind="ExternalInput")
    d_dist2 = nc.dram_tensor("dist2", [2, NEc], bf, kind="ExternalInput")
    d_out = nc.dram_tensor("out", [1, LIGc], f32, kind="ExternalOutput")
    if dbg:
        d_dbg_ea = nc.dram_tensor("dbg_ea", [128, 16384], bf, kind="ExternalOutput")
        d_dbg_oh = nc.dram_tensor("dbg_oh", [128, 8192], bf, kind="ExternalOutput")
        d_dbg_in = nc.dram_tensor("dbg_in", [32, NLc], bf, kind="ExternalOutput")
        d_dbg_hh = nc.dram_tensor("dbg_hh", [128, NLc], f32, kind="ExternalOutput")

    # ---------------- const packing (baked into NEFF) ----------------
    def aug(W):  # [K,128] -> [K,129] with col 128 = row-wise mean over outputs
        return np.concatenate([W, W.mean(1, keepdims=True)], 1)

    We1 = np.asarray(gcl_We1)  # [D, 292, 128]
    Wn1 = np.asarray(gcl_Wn1)  # [D, 256, 128]
    Aaug = np.zeros((128, DEPTH, 129), np.float32)
    Baug = np.zeros((128, DEPTH, 129), np.float32)
    Cpack = np.zeros((128, DEPTH, 129), np.float32)
    N1aug = np.zeros((128, DEPTH, 2, 129), np.float32)
    for l in range(DEPTH):
        Aaug[:, l] = aug(We1[l][0:128])
        Baug[:, l] = aug(We1[l][128:256])
        C = aug(We1[l][256:292])            # [36,129]: 0:16 emb, 16:36 smear
        Cpack[0:16, l] = C[0:16]
        Cpack[32:52, l] = C[16:36]
        Cpack[64:80, l] = C[0:16]
        Cpack[96:116, l] = C[16:36]
        N1aug[:, l, 0] = aug(Wn1[l][0:128])
        N1aug[:, l, 1] = aug(Wn1[l][128:256] / NORM_FACTOR)
    We2s = np.ascontiguousarray(np.transpose(np.asarray(gcl_We2), (1, 0, 2)))
    Watts = np.ascontiguousarray(np.transpose(np.asarray(gcl_Watt), (1, 0, 2)))
    Wn2s = np.ascontiguousarray(np.transpose(np.asarray(gcl_Wn2), (1, 0, 2)))

    Rall = np.zeros((128, 256), np.float32)
    for b in range(4):
        for hs in range(2):
            for e in range(128):
                Rall[32 * b + 16 * hs + e // 8, 128 * hs + e] = 1.0
    Spat = np.zeros((128, 256), np.float32)
    for j in range(8):
        for p in range(128):
            Spat[p, 32 * j + 16 * (j % 2) + p // 8] = 1.0
    poolpat = np.zeros((128, 4), np.float32)
    for n in range(128):
        poolpat[n, n // 32] = 1.0 / 32.0

    actc = np.zeros((128, 2), np.float32)
    actc[32:52, 0] = -GS_OFFSET
    actc[32:52, 1] = GS_COEFF
    actc[96:116, 0] = -GS_OFFSET
    actc[96:116, 1] = GS_COEFF
    rowmod = (np.arange(128) % 32).astype(np.float32).reshape(128, 1)

    def cb(a, name):
        return nc.inline_tensor(np.ascontiguousarray(a).astype(bf16), name=name)

    def cf(a, name):
        return nc.inline_tensor(np.ascontiguousarray(a).astype(np.float32),
                                name=name)

    d_A = cb(Aaug, "cA")
    d_B = cb(Baug, "cB")
    d_C = cb(Cpack, "cC")
    d_We2 = cb(We2s, "cWe2")
    d_Watt = cb(Watts, "cWatt")
    d_N1 = cb(N1aug, "cN1")
    d_Wn2 = cb(Wn2s, "cWn2")
    d_Win = cb(np.asarray(W_in), "cWin")
    d_Woe = cb(np.asarray(W_oe), "cWoe")
    d_Wf = cf(np.asarray(W_f), "cWf")
    d_R = cb(Rall, "cR")
    d_S = cb(Spat, "cS")
    d_pool = cb(poolpat, "cpool")
    d_tab = cf(np.asarray(table).T, "ctab")           # [16, 1000]
    d_ones20 = cb(np.ones((2, NG)), "cones20")
    d_ones32 = cb(np.ones((1, 32)), "cones32")
    d_actc = cf(actc, "cactc")
    d_rowmod = cf(rowmod, "crowmod")

    with tile.TileContext(nc) as tc:
        with tc.tile_pool(name="stat", bufs=1) as stat, \
             tc.tile_pool(name="hhp", bufs=1) as hhp, \
             tc.tile_pool(name="stg", bufs=4) as stg, \
             tc.tile_pool(name="sml", bufs=6) as sml, \
             tc.tile_pool(name="ps1", bufs=2, space="PSUM") as ps1, \
             tc.tile_pool(name="ps2", bufs=1, space="PSUM") as ps2, \
             tc.tile_pool(name="ps3", bufs=2, space="PSUM") as ps3:

            # ---------- persistent SBUF state ----------
            t_ea = stat.tile([128, 16384], bf, tag="t_ea")
            t_oh = stat.tile([128, 8192], bf, tag="t_oh")
            t_R = stat.tile([128, 256], bf, tag="t_R")
            nc.sync.dma_start(t_R[:], d_R[:])
            t_S = stat.tile([128, 256], bf, tag="t_S")
            nc.sync.dma_start(t_S[:], d_S[:])
            t_Aaug = stat.tile([128, DEPTH, 129], bf, tag="t_Aaug")
            nc.sync.dma_start(t_Aaug[:], d_A[:])
            t_Baug = stat.tile([128, DEPTH, 129], bf, tag="t_Baug")
            nc.sync.dma_start(t_Baug[:], d_B[:])
            t_Caug = stat.tile([128, DEPTH, 129], bf, tag="t_Caug")
            nc.sync.dma_start(t_Caug[:], d_C[:])
            t_We2 = stat.tile([128, DEPTH, 128], bf, tag="t_We2")
            nc.sync.dma_start(t_We2[:], d_We2[:])
            t_Watt = stat.tile([128, DEPTH, 1], bf, tag="t_Watt")
            nc.sync.dma_start(t_Watt[:], d_Watt[:])
            t_N1 = stat.tile([128, DEPTH, 2, 129], bf, tag="t_N1")
            nc.sync.dma_start(t_N1[:], d_N1[:])
            t_Wn2 = stat.tile([128, DEPTH, 128], bf, tag="t_Wn2")
            nc.sync.dma_start(t_Wn2[:], d_Wn2[:])
            t_Win = stat.tile([32, 128], bf, tag="t_Win")
            nc.sync.dma_start(t_Win[:], d_Win[:])
            t_Woe = stat.tile([128, 64], bf, tag="t_Woe")
            nc.sync.dma_start(t_Woe[:], d_Woe[:])
            t_pool = stat.tile([128, 4], bf, tag="t_pool")
            nc.sync.dma_start(t_pool[:], d_pool[:])
            t_Wf = stat.tile([64, 1], f32, tag="t_Wf")
            nc.sync.dma_start(t_Wf[:], d_Wf[:])
            t_ones20 = stat.tile([2, NG], bf, tag="t_ones20")
            nc.sync.dma_start(t_ones20[:], d_ones20[:])
            t_ones32 = stat.tile([1, 32], bf, tag="t_ones32")
            nc.sync.dma_start(t_ones32[:], d_ones32[:])
            t_actc = stat.tile([128, 2], f32, tag="t_actc")
            nc.sync.dma_start(t_actc[:], d_actc[:])
            t_rowmod = stat.tile([128, 1], f32, tag="t_rowmod")
            nc.sync.dma_start(t_rowmod[:], d_rowmod[:])
            t_eps = stat.tile([128, 1], f32, tag="t_eps")
            nc.vector.memset(t_eps[:], EPS)

            hh_f = hhp.tile([128, NLc], f32, tag="hh_f")
            hh_b = hhp.tile([128, NLc], bf, tag="hh_b")
            agg_fm = hhp.tile([128, NLc], bf, tag="agg_fm")
            nm_fm = hhp.tile([128, NLc], bf, tag="nm_fm")
            nodeA = hhp.tile([128, 32, 129], bf, tag="nodeA")
            nodeB = hhp.tile([128, 32, 129], bf, tag="nodeB")
            att_em = hhp.tile([128, NCHUNK], f32, tag="att_em")

            # ---------- startup: build t_ea, t_oh, hh0 on device ----------
            with tc.tile_pool(name="boot", bufs=1) as boot:
                t_in32 = boot.tile([32, NLc], bf, tag="t_in32")
                nc.sync.dma_start(t_in32[0:16, :], d_hfm[:])
                t_idx = boot.tile([128, 2304], i16, tag="t_idx")
                nc.sync.dma_start(t_idx[0:16, :], d_idx[:])
                t_tab = boot.tile([128, NT], f32, tag="t_tab")
                nc.sync.dma_start(t_tab[0:16, :], d_tab[:])

                # zero t_ea (bands 16:32 / 80:96 stay zero and get contracted
                # against zero weight rows; compute-engine partition starts
                # must be 32-aligned, so zero everything then overwrite)
                nc.vector.memset(t_ea[:], 0.0)

                # time-emb gathers (gpsimd). ap_gather only works at
                # partition base 0, so gather+convert at rows 0:16 and DMA
                # to target partitions where needed.
                for k in range(8):
                    scr = boot.tile([128, 2048], f32, tag="scr", bufs=1)
                    nc.gpsimd.ap_gather(scr[0:16, :], t_tab[0:16, :],
                                        t_idx[0:16, 128 * k:128 * k + 128],
                                        channels=16, num_elems=NT, d=1,
                                        num_idxs=2048)
                    nc.vector.tensor_copy(t_ea[0:16, 2048 * k:2048 * k + 2048],
                                          scr[0:16, :])
                for k in range(8):
                    scr = boot.tile([128, 2048], f32, tag="scr", bufs=1)
                    nc.gpsimd.ap_gather(scr[0:16, :], t_tab[0:16, :],
                                        t_idx[0:16, 1024 + 128 * k:1024 + 128 * k + 128],
                                        channels=16, num_elems=NT, d=1,
                                        num_idxs=2048)
                    scrb = boot.tile([128, 2048], bf, tag="scrb", bufs=2)
                    nc.vector.tensor_copy(scrb[0:16, :], scr[0:16, :])
                    nc.sync.dma_start(t_ea[64:80, 2048 * k:2048 * k + 2048],
                                      scrb[0:16, :])
                for k in range(2):
                    scr = boot.tile([128, 2048], f32, tag="scr", bufs=1)
                    nc.gpsimd.ap_gather(scr[0:16, :], t_tab[0:16, :],
                                        t_idx[0:16, 2048 + 128 * k:2048 + 128 * k + 128],
                                        channels=16, num_elems=NT, d=1,
                                        num_idxs=2048)
                    scrb = boot.tile([128, 2048], bf, tag="scrb", bufs=2)
                    nc.vector.tensor_copy(scrb[0:16, :], scr[0:16, :])
                    nc.sync.dma_start(t_in32[16:32, 2048 * k:2048 * k + 2048],
                                      scrb[0:16, :])

                # Gaussian smearing: smear[g,e] = exp(coeff_g*(dist_e-off_g)^2)
                # dist broadcast to 20 partitions via PE (hi+lo bf16 sum).
                for blk in range(16):
                    dv = boot.tile([2, 2048], bf, tag="dv", bufs=2)
                    nc.sync.dma_start(dv[:], d_dist2[:, 2048 * blk:2048 * blk + 2048])
                    for s in range(4):
                        i = 4 * blk + s
                        r0 = 32 if i < 32 else 96
                        cc = 512 * (i % 32)
                        ps = ps3.tile([128, 512], f32, tag="aggatt")
                        nc.tensor.matmul(ps[r0:r0 + 20, :], lhsT=t_ones20[:],
                                         rhs=dv[:, 512 * s:512 * s + 512],
                                         start=True, stop=True,
                                         tile_position=(0, r0))
                        sq = boot.tile([128, 512], f32, tag="sq", bufs=2)
                        nc.scalar.activation(sq[r0:r0 + 20, :], ps[r0:r0 + 20, :],
                                             AF.Square, bias=t_actc[r0:r0 + 20, 0:1])
                        nc.scalar.activation(t_ea[r0:r0 + 20, cc:cc + 512],
                                             sq[r0:r0 + 20, :], AF.Exp,
                                             scale=t_actc[r0:r0 + 20, 1:2])

                # col one-hot: t_oh[32b+v, 256g+r] = (colv[1024g+256b+r] == v)
                for blk in range(16):
                    cv = boot.tile([1, 2048], bf, tag="cv", bufs=2)
                    nc.sync.dma_start(cv[:], d_colv[:, 2048 * blk:2048 * blk + 2048])
                    for gi in range(2):
                        g = 2 * blk + gi
                        ps = ps3.tile([128, 512], f32, tag="aggatt")
                        for b in range(4):
                            nc.tensor.matmul(
                                ps[32 * b:32 * b + 32, 0:256], lhsT=t_ones32[:],
                                rhs=cv[:, 1024 * gi + 256 * b:1024 * gi + 256 * b + 256],
                                start=True, stop=True, tile_position=(0, 32 * b))
                        rm_bc = bass.AP(tensor=t_rowmod[:].tensor,
                                        offset=t_rowmod[:, 0:1].offset,
                                        ap=[t_rowmod[:].ap[0], [0, 256]])
                        nc.vector.tensor_tensor(
                            out=t_oh[:, 256 * g:256 * g + 256],
                            in0=ps[:, 0:256], in1=rm_bc, op=ALU.is_equal)

                # prologue: hh0 = [h|emb] @ Win
                for nb in range(8):
                    p = ps2.tile([128, 2, 512], f32, tag="v2")
                    nc.tensor.matmul(p[:, 0, :], lhsT=t_Win[:],
                                     rhs=t_in32[:, 512 * nb:512 * nb + 512],
                                     start=True, stop=True)
                    nc.scalar.activation(hh_f[:, 512 * nb:512 * nb + 512],
                                         p[:, 0, :], AF.Copy)
                    nc.vector.tensor_copy(hh_b[:, 512 * nb:512 * nb + 512],
                                          p[:, 0, :])
                if dbg:
                    nc.sync.dma_start(d_dbg_ea[:], t_ea[:])
                    nc.sync.dma_start(d_dbg_oh[:], t_oh[:])
                    nc.sync.dma_start(d_dbg_in[:], t_in32[:])
                    nc.sync.dma_start(d_dbg_hh[:], hh_f[:])

            # ---------- layers ----------
            for l in range(DEPTH):
                # nodeA/nodeB (node-major, 129 cols incl aug-mean)
                for nb in range(32):
                    pn = ps1.tile([128, 2, 512], f32, tag="m1pre")
                    nc.tensor.matmul(pn[:, 0, 0:129], lhsT=hh_b[:, 128 * nb:128 * nb + 128],
                                     rhs=t_Aaug[:, l, :], start=True, stop=True)
                    nc.tensor.matmul(pn[:, 1, 0:129], lhsT=hh_b[:, 128 * nb:128 * nb + 128],
                                     rhs=t_Baug[:, l, :], start=True, stop=True)
                    nc.scalar.activation(nodeA[:, nb, :], pn[:, 0, 0:129], AF.Copy)
                    nc.vector.tensor_copy(nodeB[:, nb, :], pn[:, 1, 0:129])

                for g in range(NGRP):
                    # ---- m1_pre: process in 2 halves of 4 chunks (2 psum tiles) ----
                    m1_em = stg.tile([128, 1024], bf, tag="m1_em")
                    m1_fm = stg.tile([128, 1024], bf, tag="m1_fm")
                    for half in range(2):
                        pts = []
                        for hh2 in range(2):
                            pt = ps1.tile([128, 2, 512], f32, tag="m1pre")
                            pts.append(pt)
                        mv4 = sml.tile([128, 4, 2], f32, tag="mv4")
                        st4 = sml.tile([128, 4, 6], f32, tag="st4")
                        for jj in range(4):
                            j = 4 * half + jj
                            c = 8 * g + j
                            L = c // 2
                            base = 32 * (L % 4)
                            hs = c % 2
                            eh = 0 if c < 128 else 1
                            pt = pts[jj // 2]
                            sl = pt[:, jj % 2, 0:129]
                            nc.tensor.matmul(sl, lhsT=t_R[base:base + 32, 128 * hs:128 * hs + 128],
                                             rhs=nodeA[base:base + 32, L // 4, :],
                                             start=True, stop=False, tile_position=(base, 0))
                            ohf = 128 * (2 * (c // 8) + hs)
                            nc.tensor.matmul(sl, lhsT=t_oh[base:base + 32, ohf:ohf + 128],
                                             rhs=nodeB[base:base + 32, L // 4, :],
                                             start=False, stop=False, tile_position=(base, 0))
                            nc.tensor.matmul(sl, lhsT=t_ea[64 * eh:64 * eh + 52, 128 * (c % 128):128 * (c % 128) + 128],
                                             rhs=t_Caug[64 * eh:64 * eh + 52, l, :],
                                             start=False, stop=True, tile_position=(64 * eh, 0))
                            nc.vector.bn_stats(st4[:, jj, :], pt[:, jj % 2, 0:128])
                            nc.vector.bn_aggr(mv4[:, jj, :], st4[:, jj, :])
                        rstd4 = sml.tile([128, 4], f32, tag="rstd4")
                        nmr4 = sml.tile([128, 4], f32, tag="nmr4")
                        nc.scalar.activation(rstd4[:], mv4[:, :, 1], AF.Sqrt, bias=t_eps[:], scale=1.0)
                        nc.vector.reciprocal(rstd4[:], rstd4[:])
                        nc.vector.scalar_tensor_tensor(nmr4[:], in0=mv4[:, :, 0], scalar=-1.0,
                                                       in1=rstd4[:], op0=ALU.mult, op1=ALU.mult)
                        for jj in range(4):
                            j = 4 * half + jj
                            pt = pts[jj // 2]
                            nc.scalar.activation(m1_em[:, 128 * j:128 * j + 128], pt[:, jj % 2, 0:128],
                                                 AF.Silu, bias=nmr4[:, jj:jj + 1], scale=rstd4[:, jj:jj + 1])
                            nc.sync.dma_start_transpose(m1_fm[:, 128 * j:128 * j + 128],
                                                        m1_em[:, 128 * j:128 * j + 128])
                    # We2 -> v2 (feature-major) + SiLU -> mij_fm bf16
                    pv2 = ps2.tile([128, 2, 512], f32, tag="v2")
                    nc.tensor.matmul(pv2[:, 0, :], lhsT=t_We2[:, l, :], rhs=m1_fm[:, 0:512],
                                     start=True, stop=True)
                    nc.tensor.matmul(pv2[:, 1, :], lhsT=t_We2[:, l, :], rhs=m1_fm[:, 512:1024],
                                     start=True, stop=True)
                    mij_fm = stg.tile([128, 1024], bf, tag="mij_fm")
                    nc.scalar.activation(mij_fm[:], pv2[:].rearrange("p a b -> p (a b)"), AF.Silu)
                    # att: edge-major [128,1] per chunk via mij_fm as lhsT
                    patt = ps3.tile([128, 512], f32, tag="aggatt")
                    for j in range(8):
                        nc.tensor.matmul(patt[:, j:j + 1], lhsT=mij_fm[:, 128 * j:128 * j + 128],
                                         rhs=t_Watt[:, l, :], start=True, stop=True)
                    nc.scalar.activation(att_em[:, 8 * g:8 * g + 8], patt[:, 0:8], AF.Sigmoid)
                    # S*att (bf16) via bcast-TT
                    satt = stg.tile([128, 256], bf, tag="satt")
                    att_bc = bass.AP(tensor=att_em[:].tensor, offset=att_em[:, 8 * g:8 * g + 8].offset,
                                     ap=[att_em[:].ap[0], [1, 8], [0, 32]])
                    nc.vector.tensor_tensor(out=satt[:].rearrange("p (a b) -> p a b", a=8),
                                            in0=t_S[:].rearrange("p (a b) -> p a b", a=8),
                                            in1=att_bc, op=ALU.mult)
                    # mij back to edge-major
                    mij_em = stg.tile([128, 1024], bf, tag="mij_em")
                    for j in range(8):
                        nc.sync.dma_start_transpose(mij_em[:, 128 * j:128 * j + 128],
                                                    mij_fm[:, 128 * j:128 * j + 128])
                    # gated segment-sum -> node-major agg [128 nodes, 128]
                    pagg = ps3.tile([128, 512], f32, tag="aggatt")
                    for j in range(8):
                        nc.tensor.matmul(pagg[32 * (j // 2):32 * (j // 2) + 32, 0:128],
                                         lhsT=satt[:, 32 * j:32 * j + 32],
                                         rhs=mij_em[:, 128 * j:128 * j + 128],
                                         start=(j % 2 == 0), stop=(j % 2 == 1),
                                         tile_position=(0, 32 * (j // 2)))
                    # evac agg (node-major bf16) then transpose to feature-major
                    agg_nm = stg.tile([128, 128], bf, tag="agg_nm")
                    nc.scalar.activation(agg_nm[:], pagg[:, 0:128], AF.Copy)
                    nc.sync.dma_start_transpose(agg_fm[:, 128 * g:128 * g + 128], agg_nm[:])

                # ---- node MLP ----
                for nb in range(16):
                    pn = ps1.tile([128, 2, 512], f32, tag="m1pre")
                    mv2 = sml.tile([128, 2, 2], f32, tag="mv2")
                    st2 = sml.tile([128, 2, 6], f32, tag="st2")
                    for s in range(2):
                        cb2 = 2 * nb + s
                        sl = pn[:, s, 0:129]
                        nc.tensor.matmul(sl, lhsT=hh_b[:, 128 * cb2:128 * cb2 + 128],
                                         rhs=t_N1[:, l, 0, :], start=True, stop=False)
                        nc.tensor.matmul(sl, lhsT=agg_fm[:, 128 * cb2:128 * cb2 + 128],
                                         rhs=t_N1[:, l, 1, :], start=False, stop=True)
                        nc.vector.bn_stats(st2[:, s, :], pn[:, s, 0:128])
                        nc.vector.bn_aggr(mv2[:, s, :], st2[:, s, :])
                    rstd2 = sml.tile([128, 2], f32, tag="rstd2")
                    nmr2 = sml.tile([128, 2], f32, tag="nmr2")
                    nc.scalar.activation(rstd2[:], mv2[:, :, 1], AF.Sqrt, bias=t_eps[:], scale=1.0)
                    nc.vector.reciprocal(rstd2[:], rstd2[:])
                    nc.vector.scalar_tensor_tensor(nmr2[:], in0=mv2[:, :, 0], scalar=-1.0,
                                                   in1=rstd2[:], op0=ALU.mult, op1=ALU.mult)
                    nm_nm = stg.tile([128, 256], bf, tag="nm_nm")
                    for s in range(2):
                        cb2 = 2 * nb + s
                        nc.scalar.activation(nm_nm[:, 128 * s:128 * s + 128], pn[:, s, 0:128],
                                             AF.Silu, bias=nmr2[:, s:s + 1], scale=rstd2[:, s:s + 1])
                        nc.sync.dma_start_transpose(nm_fm[:, 128 * cb2:128 * cb2 + 128],
                                                    nm_nm[:, 128 * s:128 * s + 128])
                # hh update: hh += nm @ Wn2
                for nb in range(8):
                    pu = ps2.tile([128, 2, 512], f32, tag="v2")
                    nc.tensor.matmul(pu[:, 0, :], lhsT=t_Wn2[:, l, :],
                                     rhs=nm_fm[:, 512 * nb:512 * nb + 512], start=True, stop=True)
                    nc.vector.tensor_add(hh_f[:, 512 * nb:512 * nb + 512],
                                         hh_f[:, 512 * nb:512 * nb + 512], pu[:, 0, :])
                    nc.vector.tensor_copy(hh_b[:, 512 * nb:512 * nb + 512],
                                          hh_f[:, 512 * nb:512 * nb + 512])

            # ---------- epilogue: ho = hh @ Woe, ligand mean-pool, @ Wf ----------
            pooled_ps = ps3.tile([128, 512], f32, tag="aggatt")
            for nb in range(32):
                ph = ps1.tile([128, 2, 512], f32, tag="m1pre")
                nc.tensor.matmul(ph[:, 0, 0:64], lhsT=hh_b[:, 128 * nb:128 * nb + 128],
                                 rhs=t_Woe[:], start=True, stop=True)
                ho_nm = stg.tile([128, 64], bf, tag="ho_nm")
                nc.scalar.activation(ho_nm[:], ph[:, 0, 0:64], AF.Copy)
                nc.tensor.matmul(pooled_ps[0:64, 4 * nb:4 * nb + 4], lhsT=ho_nm[:],
                                 rhs=t_pool[:], start=True, stop=True)
            pooled_sb = stat.tile([64, 128], f32, tag="pooled_sb")
            nc.vector.tensor_copy(pooled_sb[:], pooled_ps[0:64, 0:128])
            pfin = ps3.tile([128, 512], f32, tag="aggatt")
            nc.tensor.matmul(pfin[0:1, 0:128], lhsT=t_Wf[:], rhs=pooled_sb[:],
                             start=True, stop=True)
            out_sb = stat.tile([1, 128], f32, tag="out_sb")
            nc.vector.tensor_copy(out_sb[:], pfin[0:1, 0:128])
            nc.sync.dma_start(d_out[:], out_sb[:])

    nc.compile()
    return nc


def _prep_inputs(x, h, t, edges, t_bond, batch_ligand, time_emb_table,
                 W_in, gcl_We1, gcl_Wn1, gcl_We2, gcl_Watt, gcl_Wn2,
                 W_oe, W_f):
    """Host-side sharding of dynamic inputs. Returns list of in_maps."""
    row = np.asarray(edges[0])
    col = np.asarray(edges[1])
    assert np.array_equal(row, np.repeat(np.arange(N), KNN)), "row structure"
    assert np.array_equal(np.asarray(batch_ligand), np.arange(N) // K), "batch structure"
    assert np.all(col // K == row // K), "edges cross ligands"

    # edge time-bond indices (host index arithmetic)
    sbi = row * (K - 1) + col - (row // K) * K - (row < col).astype(row.dtype)
    tbe = np.asarray(t_bond)[sbi].astype(np.int16)        # [E]
    tn = np.asarray(t).astype(np.int16)                   # [N]
    # distances (host)
    xx = np.asarray(x)
    cdiff = xx[row] - xx[col]
    radial = (cdiff ** 2).sum(1)
    dist = np.clip(np.sqrt(radial), 0.0, 4.0).astype(np.float32)
    dhi = dist.astype(bf16)
    dlo = (dist - dhi.astype(np.float32)).astype(bf16)
    col_loc = (col % K).astype(np.float32)

    hh = np.asarray(h)
    maps = []
    for ci in range(NCORES):
        n0 = ci * NLc
        e0 = ci * NEc
        tbe_c = tbe[e0:e0 + NEc]
        idxw0 = np.ascontiguousarray(tbe_c[:16384].reshape(1024, 16).T)
        idxw1 = np.ascontiguousarray(tbe_c[16384:].reshape(1024, 16).T)
        tw = np.ascontiguousarray(tn[n0:n0 + NLc].reshape(256, 16).T)
        idx_all = np.concatenate([idxw0, idxw1, tw], axis=1)  # [16, 2304]
        m = dict(
            hfm=np.ascontiguousarray(hh[n0:n0 + NLc].T).astype(bf16),
            idxs=np.ascontiguousarray(idx_all),
            colv=np.ascontiguousarray(col_loc[e0:e0 + NEc].reshape(1, NEc)).astype(bf16),
            dist2=np.ascontiguousarray(
                np.stack([dhi[e0:e0 + NEc], dlo[e0:e0 + NEc]])),
        )
        maps.append(m)
    return maps


def kernel(x, h, t, edges, t_bond, batch_ligand, num_atoms_per_ligand,
           num_ligands, time_emb_table, W_in, b_in, gcl_We1, gcl_be1, gcl_g1,
           gcl_bt1, gcl_We2, gcl_be2, gcl_Watt, gcl_batt, gcl_Wn1, gcl_bn1,
           gcl_g2, gcl_bt2, gcl_Wn2, gcl_bn2, W_oe, b_oe, W_f, b_f):
    from concourse.bass_utils import run_bass_kernel_spmd

    # all biases zero / gains one in this model family; verify then fold away
    for z in (b_in, gcl_be1, gcl_bt1, gcl_be2, gcl_batt, gcl_bn1, gcl_bt2,
              gcl_bn2, b_oe, b_f):
        assert np.abs(np.asarray(z)).max() == 0.0, "nonzero bias unsupported"
    for o in (gcl_g1, gcl_g2):
        assert np.abs(np.asarray(o) - 1.0).max() == 0.0, "non-unit LN gain"
    assert int(num_atoms_per_ligand) == K and int(num_ligands) == N_LIG

    wts = (W_in, gcl_We1, gcl_Wn1, gcl_We2, gcl_Watt, gcl_Wn2, W_oe, W_f,
           time_emb_table)
    import hashlib
    hsh = hashlib.sha1()
    for w in wts:
        hsh.update(np.ascontiguousarray(np.asarray(w, np.float32)).tobytes())
    key = hsh.hexdigest()
    if _COMPILED.get("key") != key:
        _COMPILED["prog"] = _build_program(*wts)
        _COMPILED["key"] = key
    nc = _COMPILED["prog"]

    maps = _prep_inputs(x, h, t, edges, t_bond, batch_ligand, time_emb_table,
                        W_in, gcl_We1, gcl_Wn1, gcl_We2, gcl_Watt, gcl_Wn2,
                        W_oe, W_f)
    res = run_bass_kernel_spmd(nc, maps, list(range(NCORES)))
    out = np.concatenate([r["out"][0] for r in res.results])
    return out.astype(np.float32)


# revision 20
# speedup vs baseline: 19.1771x; 12.3750x over previous
"""GNN message-passing (EGNN-style classifier) on 8 TRN2 NeuronCores.

Data-parallel over ligands: each core handles 128 ligands = 4096 nodes,
32768 edges (edges never cross ligands). Weights replicated.

The axon tunnel moves ~25 MB/s, so the per-dispatch host->device payload
dominates wall clock. All weights and static gather patterns are baked into
the NEFF as Const tensors (shipped once at load); the per-dispatch inputs are
only the true dynamic data (~392 KB/core):
  - hfm   [16,4096]  bf16  node features h, feature-major
  - idxs  [16,2304]  i16   time-emb table indices (edge t_bond + node t),
                           wrapped for gpsimd ap_gather
  - colv  [1,32768]  bf16  col atom index within ligand (0..31)
  - dist2 [2,32768]  bf16  edge distance as hi/lo bf16 pair (sums to f32 dist)
Edge attributes (time-emb gather + Gaussian smearing) and the col one-hot
gather matrix are built on device at startup.

Device layout (per core):
- Node state hh kept feature-major [128 feats, 4096 nodes] in SBUF (f32 master
  + bf16 copy for matmul inputs).
- Edge pipeline per layer, per group of 1024 edges (8 chunks x 128 edges):
  m1_pre = hh[row] @ A + hh[col] @ B + edge_attr @ C computed edge-major via
  three PE matmuls per chunk (R-gather / one-hot gather / edge-attr lhsT).
  LayerNorm stats via DVE bn_stats on PSUM; fused scale/bias+SiLU on ACT.
  DMA-transpose to feature-major, We2 matmul, SiLU, attention via PE,
  gated segment-sum via one-hot-weighted (S*att) matmuls back to node-major.
- Node MLP node-major with the same LN trick; residual update in f32.
- t_ea rows {0:16 emb, 16:32 zero, 32:52 smear} for edges 0:16384 and
  {64:80, 80:96, 96:116} for edges 16384:32768; C weight packed to match.
"""
import numpy as np
import ml_dtypes

N_LIG = 1024
K = 32                 # atoms per ligand
N = N_LIG * K          # 32768 nodes
KNN = 8
E = N * KNN            # 262144 edges
IN_F = 16
T_F = 16
HID = 128
OUT_F = 64
DEPTH = 4
NG = 20
NT = 1000
EDGE_IN = NG + T_F
NORM_FACTOR = 5.0
EPS = 1e-5

NCORES = 8
NLc = N // NCORES      # 4096 nodes / core
NEc = E // NCORES      # 32768 edges / core
LIGc = N_LIG // NCORES  # 128 ligands / core
NCHUNK = NEc // 128    # 256 edge chunks / core
NGRP = NCHUNK // 8     # 32 groups of 1024 edges

bf16 = ml_dtypes.bfloat16

# Gaussian smearing constants
_off = np.exp(np.linspace(np.log(1.0), np.log(5.0), NG)) - 1.0
_d = np.diff(_off)
_d = np.concatenate([_d[:1], _d])
GS_OFFSET = _off.astype(np.float32)
GS_COEFF = (-0.5 / _d ** 2).astype(np.float32)

_COMPILED = {}


def _build_program(W_in, gcl_We1, gcl_Wn1, gcl_We2, gcl_Watt, gcl_Wn2,
                   W_oe, W_f, table, dbg=False):
    import concourse.bacc as bacc
    import concourse.bass as bass
    import concourse.mybir as mybir
    import concourse.tile as tile

    bf = mybir.dt.bfloat16
    f32 = mybir.dt.float32
    i16 = mybir.dt.int16
    AF = mybir.ActivationFunctionType
    ALU = mybir.AluOpType

    nc = bacc.Bacc("TRN2", target_bir_lowering=False, debug=False)

    # ---------------- dynamic inputs (per dispatch) ----------------
    d_hfm = nc.dram_tensor("hfm", [16, NLc], bf, kind="ExternalInput")
    d_idx = nc.dram_tensor("idxs", [16, 2304], i16, kind="ExternalInput")
    d_colv = nc.dram_tensor("colv", [1, NEc], bf, kind="ExternalInput")
    d_dist2 = nc.dram_tensor("dist2", [2, NEc], bf, kind="ExternalInput")
    d_out = nc.dram_tensor("out", [1, LIGc], f32, kind="ExternalOutput")
    if dbg:
        d_dbg_ea = nc.dram_tensor("dbg_ea", [128, 16384], bf, kind="ExternalOutput")
        d_dbg_oh = nc.dram_tensor("dbg_oh", [128, 8192], bf, kind="ExternalOutput")
        d_dbg_in = nc.dram_tensor("dbg_in", [32, NLc], bf, kind="ExternalOutput")
        d_dbg_hh = nc.dram_tensor("dbg_hh", [128, NLc], f32, kind="ExternalOutput")

    # ---------------- const packing (baked into NEFF) ----------------
    def aug(W):  # [K,128] -> [K,129] with col 128 = row-wise mean over outputs
        return np.concatenate([W, W.mean(1, keepdims=True)], 1)

    We1 = np.asarray(gcl_We1)  # [D, 292, 128]
    Wn1 = np.asarray(gcl_Wn1)  # [D, 256, 128]
    Aaug = np.zeros((128, DEPTH, 129), np.float32)
    Baug = np.zeros((128, DEPTH, 129), np.float32)
    Cpack = np.zeros((128, DEPTH, 129), np.float32)
    N1aug = np.zeros((128, DEPTH, 2, 129), np.float32)
    for l in range(DEPTH):
        Aaug[:, l] = aug(We1[l][0:128])
        Baug[:, l] = aug(We1[l][128:256])
        C = aug(We1[l][256:292])            # [36,129]: 0:16 emb, 16:36 smear
        Cpack[0:16, l] = C[0:16]
        Cpack[32:52, l] = C[16:36]
        Cpack[64:80, l] = C[0:16]
        Cpack[96:116, l] = C[16:36]
        N1aug[:, l, 0] = aug(Wn1[l][0:128])
        N1aug[:, l, 1] = aug(Wn1[l][128:256] / NORM_FACTOR)
    We2s = np.ascontiguousarray(np.transpose(np.asarray(gcl_We2), (1, 0, 2)))
    Watts = np.ascontiguousarray(np.transpose(np.asarray(gcl_Watt), (1, 0, 2)))
    Wn2s = np.ascontiguousarray(np.transpose(np.asarray(gcl_Wn2), (1, 0, 2)))

    Rall = np.zeros((128, 256), np.float32)
    for b in range(4):
        for hs in range(2):
            for e in range(128):
                Rall[32 * b + 16 * hs + e // 8, 128 * hs + e] = 1.0
    Spat = np.zeros((128, 256), np.float32)
    for j in range(8):
        for p in range(128):
            Spat[p, 32 * j + 16 * (j % 2) + p // 8] = 1.0
    poolpat = np.zeros((128, 4), np.float32)
    for n in range(128):
        poolpat[n, n // 32] = 1.0 / 32.0

    actc = np.zeros((128, 2), np.float32)
    actc[32:52, 0] = -GS_OFFSET
    actc[32:52, 1] = GS_COEFF
    actc[96:116, 0] = -GS_OFFSET
    actc[96:116, 1] = GS_COEFF
    rowmod = (np.arange(128) % 32).astype(np.float32).reshape(128, 1)

    def cb(a, name):
        return nc.inline_tensor(np.ascontiguousarray(a).astype(bf16), name=name)

    def cf(a, name):
        return nc.inline_tensor(np.ascontiguousarray(a).astype(np.float32),
                                name=name)

    d_A = cb(Aaug, "cA")
    d_B = cb(Baug, "cB")
    d_C = cb(Cpack, "cC")
    d_We2 = cb(We2s, "cWe2")
    d_Watt = cb(Watts, "cWatt")
    d_N1 = cb(N1aug, "cN1")
    d_Wn2 = cb(Wn2s, "cWn2")
    d_Win = cb(np.asarray(W_in), "cWin")
    d_Woe = cb(np.asarray(W_oe), "cWoe")
    d_Wf = cf(np.asarray(W_f), "cWf")
    d_R = cb(Rall, "cR")
    d_S = cb(Spat, "cS")
    d_pool = cb(poolpat, "cpool")
    d_tab = cf(np.asarray(table).T, "ctab")           # [16, 1000]
    d_ones20 = cb(np.ones((2, NG)), "cones20")
    d_ones32 = cb(np.ones((1, 32)), "cones32")
    d_actc = cf(actc, "cactc")
    d_rowmod = cf(rowmod, "crowmod")

    with tile.TileContext(nc) as tc:
        with tc.tile_pool(name="stat", bufs=1) as stat, \
             tc.tile_pool(name="hhp", bufs=1) as hhp, \
             tc.tile_pool(name="stg", bufs=4) as stg, \
             tc.tile_pool(name="sml", bufs=6) as sml, \
             tc.tile_pool(name="ps1", bufs=2, space="PSUM") as ps1, \
             tc.tile_pool(name="ps2", bufs=1, space="PSUM") as ps2, \
             tc.tile_pool(name="ps3", bufs=2, space="PSUM") as ps3:

            # ---------- persistent SBUF state ----------
            t_ea = stat.tile([128, 16384], bf, tag="t_ea")
            t_oh = stat.tile([128, 8192], bf, tag="t_oh")
            t_R = stat.tile([128, 256], bf, tag="t_R")
            nc.sync.dma_start(t_R[:], d_R[:])
            t_S = stat.tile([128, 256], bf, tag="t_S")
            nc.sync.dma_start(t_S[:], d_S[:])
            t_Aaug = stat.tile([128, DEPTH, 129], bf, tag="t_Aaug")
            nc.sync.dma_start(t_Aaug[:], d_A[:])
            t_Baug = stat.tile([128, DEPTH, 129], bf, tag="t_Baug")
            nc.sync.dma_start(t_Baug[:], d_B[:])
            t_Caug = stat.tile([128, DEPTH, 129], bf, tag="t_Caug")
            nc.sync.dma_start(t_Caug[:], d_C[:])
            t_We2 = stat.tile([128, DEPTH, 128], bf, tag="t_We2")
            nc.sync.dma_start(t_We2[:], d_We2[:])
            t_Watt = stat.tile([128, DEPTH, 1], bf, tag="t_Watt")
            nc.sync.dma_start(t_Watt[:], d_Watt[:])
            t_N1 = stat.tile([128, DEPTH, 2, 129], bf, tag="t_N1")
            nc.sync.dma_start(t_N1[:], d_N1[:])
            t_Wn2 = stat.tile([128, DEPTH, 128], bf, tag="t_Wn2")
            nc.sync.dma_start(t_Wn2[:], d_Wn2[:])
            t_Win = stat.tile([32, 128], bf, tag="t_Win")
            nc.sync.dma_start(t_Win[:], d_Win[:])
            t_Woe = stat.tile([128, 64], bf, tag="t_Woe")
            nc.sync.dma_start(t_Woe[:], d_Woe[:])
            t_pool = stat.tile([128, 4], bf, tag="t_pool")
            nc.sync.dma_start(t_pool[:], d_pool[:])
            t_Wf = stat.tile([64, 1], f32, tag="t_Wf")
            nc.sync.dma_start(t_Wf[:], d_Wf[:])
            t_ones20 = stat.tile([2, NG], bf, tag="t_ones20")
            nc.sync.dma_start(t_ones20[:], d_ones20[:])
            t_ones32 = stat.tile([1, 32], bf, tag="t_ones32")
            nc.sync.dma_start(t_ones32[:], d_ones32[:])
            t_actc = stat.tile([128, 2], f32, tag="t_actc")
            nc.sync.dma_start(t_actc[:], d_actc[:])
            t_rowmod = stat.tile([128, 1], f32, tag="t_rowmod")
            nc.sync.dma_start(t_rowmod[:], d_rowmod[:])
            t_eps = stat.tile([128, 1], f32, tag="t_eps")
            nc.vector.memset(t_eps[:], EPS)

            hh_f = hhp.tile([128, NLc], f32, tag="hh_f")
            hh_b = hhp.tile([128, NLc], bf, tag="hh_b")
            agg_fm = hhp.tile([128, NLc], bf, tag="agg_fm")
            nm_fm = hhp.tile([128, NLc], bf, tag="nm_fm")
            nodeA = hhp.tile([128, 32, 129], bf, tag="nodeA")
            nodeB = hhp.tile([128, 32, 129], bf, tag="nodeB")
            att_em = hhp.tile([128, NCHUNK], f32, tag="att_em")

            # ---------- startup: build t_ea, t_oh, hh0 on device ----------
            with tc.tile_pool(name="boot", bufs=1) as boot:
                t_in32 = boot.tile([32, NLc], bf, tag="t_in32")
                nc.sync.dma_start(t_in32[0:16, :], d_hfm[:])
                t_idx = boot.tile([128, 2304], i16, tag="t_idx")
                nc.sync.dma_start(t_idx[0:16, :], d_idx[:])
                t_tab = boot.tile([128, NT], f32, tag="t_tab")
                nc.sync.dma_start(t_tab[0:16, :], d_tab[:])

                # zero t_ea (bands 16:32 / 80:96 stay zero and get contracted
                # against zero weight rows; compute-engine partition starts
                # must be 32-aligned, so zero everything then overwrite)
                nc.vector.memset(t_ea[:], 0.0)

                # time-emb gathers (gpsimd). ap_gather only works at
                # partition base 0, so gather+convert at rows 0:16 and DMA
                # to target partitions where needed.
                for k in range(8):
                    scr = boot.tile([128, 2048], f32, tag="scr", bufs=1)
                    nc.gpsimd.ap_gather(scr[0:16, :], t_tab[0:16, :],
                                        t_idx[0:16, 128 * k:128 * k + 128],
                                        channels=16, num_elems=NT, d=1,
                                        num_idxs=2048)
                    nc.vector.tensor_copy(t_ea[0:16, 2048 * k:2048 * k + 2048],
                                          scr[0:16, :])
                for k in range(8):
                    scr = boot.tile([128, 2048], f32, tag="scr", bufs=1)
                    nc.gpsimd.ap_gather(scr[0:16, :], t_tab[0:16, :],
                                        t_idx[0:16, 1024 + 128 * k:1024 + 128 * k + 128],
                                        channels=16, num_elems=NT, d=1,
                                        num_idxs=2048)
                    scrb = boot.tile([128, 2048], bf, tag="scrb", bufs=2)
                    nc.vector.tensor_copy(scrb[0:16, :], scr[0:16, :])
                    nc.sync.dma_start(t_ea[64:80, 2048 * k:2048 * k + 2048],
                                      scrb[0:16, :])
                for k in range(2):
                    scr = boot.tile([128, 2048], f32, tag="scr", bufs=1)
                    nc.gpsimd.ap_gather(scr[0:16, :], t_tab[0:16, :],
                                        t_idx[0:16, 2048 + 128 * k:2048 + 128 * k + 128],
                                        channels=16, num_elems=NT, d=1,
                                        num_idxs=2048)
                    scrb = boot.tile([128, 2048], bf, tag="scrb", bufs=2)
                    nc.vector.tensor_copy(scrb[0:16, :], scr[0:16, :])
                    nc.sync.dma_start(t_in32[16:32, 2048 * k:2048 * k + 2048],
                                      scrb[0:16, :])

                # Gaussian smearing: smear[g,e] = exp(coeff_g*(dist_e-off_g)^2)
                # dist broadcast to 20 partitions via PE (hi+lo bf16 sum).
                for blk in range(16):
                    dv = boot.tile([2, 2048], bf, tag="dv", bufs=2)
                    nc.sync.dma_start(dv[:], d_dist2[:, 2048 * blk:2048 * blk + 2048])
                    for s in range(4):
                        i = 4 * blk + s
                        r0 = 32 if i < 32 else 96
                        cc = 512 * (i % 32)
                        ps = ps3.tile([128, 512], f32, tag="aggatt")
                        nc.tensor.matmul(ps[r0:r0 + 20, :], lhsT=t_ones20[:],
                                         rhs=dv[:, 512 * s:512 * s + 512],
                                         start=True, stop=True,
                                         tile_position=(0, r0))
                        sq = boot.tile([128, 512], f32, tag="sq", bufs=2)
                        nc.scalar.activation(sq[r0:r0 + 20, :], ps[r0:r0 + 20, :],
                                             AF.Square, bias=t_actc[r0:r0 + 20, 0:1])
                        nc.scalar.activation(t_ea[r0:r0 + 20, cc:cc + 512],
                                             sq[r0:r0 + 20, :], AF.Exp,
                                             scale=t_actc[r0:r0 + 20, 1:2])

                # col one-hot: t_oh[32b+v, 256g+r] = (colv[1024g+256b+r] == v)
                for blk in range(16):
                    cv = boot.tile([1, 2048], bf, tag="cv", bufs=2)
                    nc.sync.dma_start(cv[:], d_colv[:, 2048 * blk:2048 * blk + 2048])
                    for gi in range(2):
                        g = 2 * blk + gi
                        ps = ps3.tile([128, 512], f32, tag="aggatt")
                        for b in range(4):
                            nc.tensor.matmul(
                                ps[32 * b:32 * b + 32, 0:256], lhsT=t_ones32[:],
                                rhs=cv[:, 1024 * gi + 256 * b:1024 * gi + 256 * b + 256],
                                start=True, stop=True, tile_position=(0, 32 * b))
                        rm_bc = bass.AP(tensor=t_rowmod[:].tensor,
                                        offset=t_rowmod[:, 0:1].offset,
                                        ap=[t_rowmod[:].ap[0], [0, 256]])
                        nc.vector.tensor_tensor(
                            out=t_oh[:, 256 * g:256 * g + 256],
                            in0=ps[:, 0:256], in1=rm_bc, op=ALU.is_equal)

                # prologue: hh0 = [h|emb] @ Win
                for nb in range(8):
                    p = ps2.tile([128, 2, 512], f32, tag="v2")
                    nc.tensor.matmul(p[:, 0, :], lhsT=t_Win[:],
                                     rhs=t_in32[:, 512 * nb:512 * nb + 512],
                                     start=True, stop=True)
                    nc.scalar.activation(hh_f[:, 512 * nb:512 * nb + 512],
                                         p[:, 0, :], AF.Copy)
                    nc.vector.tensor_copy(hh_b[:, 512 * nb:512 * nb + 512],
                                          p[:, 0, :])
                if dbg:
                    nc.sync.dma_start(d_dbg_ea[:], t_ea[:])
                    nc.sync.dma_start(d_dbg_oh[:], t_oh[:])
                    nc.sync.dma_start(d_dbg_in[:], t_in32[:])
                    nc.sync.dma_start(d_dbg_hh[:], hh_f[:])

            # ---------- layers ----------
            for l in range(DEPTH):
                # nodeA/nodeB (node-major, 129 cols incl aug-mean)
                for nb in range(32):
                    pn = ps1.tile([128, 2, 512], f32, tag="m1pre")
                    nc.tensor.matmul(pn[:, 0, 0:129], lhsT=hh_b[:, 128 * nb:128 * nb + 128],
                                     rhs=t_Aaug[:, l, :], start=True, stop=True)
                    nc.tensor.matmul(pn[:, 1, 0:129], lhsT=hh_b[:, 128 * nb:128 * nb + 128],
                                     rhs=t_Baug[:, l, :], start=True, stop=True)
                    nc.scalar.activation(nodeA[:, nb, :], pn[:, 0, 0:129], AF.Copy)
                    nc.vector.tensor_copy(nodeB[:, nb, :], pn[:, 1, 0:129])

                for g in range(NGRP):
                    # ---- m1_pre: process in 2 halves of 4 chunks (2 psum tiles) ----
                    m1_em = stg.tile([128, 1024], bf, tag="m1_em")
                    m1_fm = stg.tile([128, 1024], bf, tag="m1_fm")
                    for half in range(2):
                        pts = []
                        for hh2 in range(2):
                            pt = ps1.tile([128, 2, 512], f32, tag="m1pre")
                            pts.append(pt)
                        mv4 = sml.tile([128, 4, 2], f32, tag="mv4")
                        st4 = sml.tile([128, 4, 6], f32, tag="st4")
                        for jj in range(4):
                            j = 4 * half + jj
                            c = 8 * g + j
                            L = c // 2
                            base = 32 * (L % 4)
                            hs = c % 2
                            eh = 0 if c < 128 else 1
                            pt = pts[jj // 2]
                            sl = pt[:, jj % 2, 0:129]
                            nc.tensor.matmul(sl, lhsT=t_R[base:base + 32, 128 * hs:128 * hs + 128],
                                             rhs=nodeA[base:base + 32, L // 4, :],
                                             start=True, stop=False, tile_position=(base, 0))
                            ohf = 128 * (2 * (c // 8) + hs)
                            nc.tensor.matmul(sl, lhsT=t_oh[base:base + 32, ohf:ohf + 128],
                                             rhs=nodeB[base:base + 32, L // 4, :],
                                             start=False, stop=False, tile_position=(base, 0))
                            nc.tensor.matmul(sl, lhsT=t_ea[64 * eh:64 * eh + 52, 128 * (c % 128):128 * (c % 128) + 128],
                                             rhs=t_Caug[64 * eh:64 * eh + 52, l, :],
                                             start=False, stop=True, tile_position=(64 * eh, 0))
                            nc.vector.bn_stats(st4[:, jj, :], pt[:, jj % 2, 0:128])
                            nc.vector.bn_aggr(mv4[:, jj, :], st4[:, jj, :])
                        rstd4 = sml.tile([128, 4], f32, tag="rstd4")
                        nmr4 = sml.tile([128, 4], f32, tag="nmr4")
                        nc.scalar.activation(rstd4[:], mv4[:, :, 1], AF.Sqrt, bias=t_eps[:], scale=1.0)
                        nc.vector.reciprocal(rstd4[:], rstd4[:])
                        nc.vector.scalar_tensor_tensor(nmr4[:], in0=mv4[:, :, 0], scalar=-1.0,
                                                       in1=rstd4[:], op0=ALU.mult, op1=ALU.mult)
                        for jj in range(4):
                            j = 4 * half + jj
                            pt = pts[jj // 2]
                            nc.scalar.activation(m1_em[:, 128 * j:128 * j + 128], pt[:, jj % 2, 0:128],
                                                 AF.Silu, bias=nmr4[:, jj:jj + 1], scale=rstd4[:, jj:jj + 1])
                            nc.sync.dma_start_transpose(m1_fm[:, 128 * j:128 * j + 128],
                                                        m1_em[:, 128 * j:128 * j + 128])
                    # We2 -> v2 (feature-major) + SiLU -> mij_fm bf16
                    pv2 = ps2.tile([128, 2, 512], f32, tag="v2")
                    nc.tensor.matmul(pv2[:, 0, :], lhsT=t_We2[:, l, :], rhs=m1_fm[:, 0:512],
                                     start=True, stop=True)
                    nc.tensor.matmul(pv2[:, 1, :], lhsT=t_We2[:, l, :], rhs=m1_fm[:, 512:1024],
                                     start=True, stop=True)
                    mij_fm = stg.tile([128, 1024], bf, tag="mij_fm")
                    nc.scalar.activation(mij_fm[:], pv2[:].rearrange("p a b -> p (a b)"), AF.Silu)
                    # att: edge-major [128,1] per chunk via mij_fm as lhsT
                    patt = ps3.tile([128, 512], f32, tag="aggatt")
                    for j in range(8):
                        nc.tensor.matmul(patt[:, j:j + 1], lhsT=mij_fm[:, 128 * j:128 * j + 128],
                                         rhs=t_Watt[:, l, :], start=True, stop=True)
                    nc.scalar.activation(att_em[:, 8 * g:8 * g + 8], patt[:, 0:8], AF.Sigmoid)
                    # S*att (bf16) via bcast-TT
                    satt = stg.tile([128, 256], bf, tag="satt")
                    att_bc = bass.AP(tensor=att_em[:].tensor, offset=att_em[:, 8 * g:8 * g + 8].offset,
                                     ap=[att_em[:].ap[0], [1, 8], [0, 32]])
                    nc.vector.tensor_tensor(out=satt[:].rearrange("p (a b) -> p a b", a=8),
                                            in0=t_S[:].rearrange("p (a b) -> p a b", a=8),
                                            in1=att_bc, op=ALU.mult)
                    # mij back to edge-major
                    mij_em = stg.tile([128, 1024], bf, tag="mij_em")
                    for j in range(8):
                        nc.sync.dma_start_transpose(mij_em[:, 128 * j:128 * j + 128],
                                                    mij_fm[:, 128 * j:128 * j + 128])
                    # gated segment-sum -> node-major agg [128 nodes, 128]
                    pagg = ps3.tile([128, 512], f32, tag="aggatt")
                    for j in range(8):
                        nc.tensor.matmul(pagg[32 * (j // 2):32 * (j // 2) + 32, 0:128],
                                         lhsT=satt[:, 32 * j:32 * j + 32],
                                         rhs=mij_em[:, 128 * j:128 * j + 128],
                                         start=(j % 2 == 0), stop=(j % 2 == 1),
                                         tile_position=(0, 32 * (j // 2)))
                    # evac agg (node-major bf16) then transpose to feature-major
                    agg_nm = stg.tile([128, 128], bf, tag="agg_nm")
                    nc.scalar.activation(agg_nm[:], pagg[:, 0:128], AF.Copy)
                    nc.sync.dma_start_transpose(agg_fm[:, 128 * g:128 * g + 128], agg_nm[:])

                # ---- node MLP ----
                for nb in range(16):
                    pn = ps1.tile([128, 2, 512], f32, tag="m1pre")
                    mv2 = sml.tile([128, 2, 2], f32, tag="mv2")
                    st2 = sml.tile([128, 2, 6], f32, tag="st2")
                    for s in range(2):
                        cb2 = 2 * nb + s
                        sl = pn[:, s, 0:129]
                        nc.tensor.matmul(sl, lhsT=hh_b[:, 128 * cb2:128 * cb2 + 128],
                                         rhs=t_N1[:, l, 0, :], start=True, stop=False)
                        nc.tensor.matmul(sl, lhsT=agg_fm[:, 128 * cb2:128 * cb2 + 128],
                                         rhs=t_N1[:, l, 1, :], start=False, stop=True)
                        nc.vector.bn_stats(st2[:, s, :], pn[:, s, 0:128])
                        nc.vector.bn_aggr(mv2[:, s, :], st2[:, s, :])
                    rstd2 = sml.tile([128, 2], f32, tag="rstd2")
                    nmr2 = sml.tile([128, 2], f32, tag="nmr2")
                    nc.scalar.activation(rstd2[:], mv2[:, :, 1], AF.Sqrt, bias=t_eps[:], scale=1.0)
                    nc.vector.reciprocal(rstd2[:], rstd2[:])
                    nc.vector.scalar_tensor_tensor(nmr2[:], in0=mv2[:, :, 0], scalar=-1.0,
                                                   in1=rstd2[:], op0=ALU.mult, op1=ALU.mult)
                    nm_nm = stg.tile([128, 256], bf, tag="nm_nm")
                    for s in range(2):
                        cb2 = 2 * nb + s
                        nc.scalar.activation(nm_nm[:, 128 * s:128 * s + 128], pn[:, s, 0:128],
                                             AF.Silu, bias=nmr2[:, s:s + 1], scale=rstd2[:, s:s + 1])
                        nc.sync.dma_start_transpose(nm_fm[:, 128 * cb2:128 * cb2 + 128],
                                                    nm_nm[:, 128 * s:128 * s + 128])
                # hh update: hh += nm @ Wn2
                for nb in range(8):
                    pu = ps2.tile([128, 2, 512], f32, tag="v2")
                    nc.tensor.matmul(pu[:, 0, :], lhsT=t_Wn2[:, l, :],
                                     rhs=nm_fm[:, 512 * nb:512 * nb + 512], start=True, stop=True)
                    nc.vector.tensor_add(hh_f[:, 512 * nb:512 * nb + 512],
                                         hh_f[:, 512 * nb:512 * nb + 512], pu[:, 0, :])
                    nc.vector.tensor_copy(hh_b[:, 512 * nb:512 * nb + 512],
                                          hh_f[:, 512 * nb:512 * nb + 512])

            # ---------- epilogue: ho = hh @ Woe, ligand mean-pool, @ Wf ----------
            pooled_ps = ps3.tile([128, 512], f32, tag="aggatt")
            for nb in range(32):
                ph = ps1.tile([128, 2, 512], f32, tag="m1pre")
                nc.tensor.matmul(ph[:, 0, 0:64], lhsT=hh_b[:, 128 * nb:128 * nb + 128],
                                 rhs=t_Woe[:], start=True, stop=True)
                ho_nm = stg.tile([128, 64], bf, tag="ho_nm")
                nc.scalar.activation(ho_nm[:], ph[:, 0, 0:64], AF.Copy)
                nc.tensor.matmul(pooled_ps[0:64, 4 * nb:4 * nb + 4], lhsT=ho_nm[:],
                                 rhs=t_pool[:], start=True, stop=True)
            pooled_sb = stat.tile([64, 128], f32, tag="pooled_sb")
            nc.vector.tensor_copy(pooled_sb[:], pooled_ps[0:64, 0:128])
            pfin = ps3.tile([128, 512], f32, tag="aggatt")
            nc.tensor.matmul(pfin[0:1, 0:128], lhsT=t_Wf[:], rhs=pooled_sb[:],
                             start=True, stop=True)
            out_sb = stat.tile([1, 128], f32, tag="out_sb")
            nc.vector.tensor_copy(out_sb[:], pfin[0:1, 0:128])
            nc.sync.dma_start(d_out[:], out_sb[:])

    nc.compile()
    return nc


def _make_dispatch(nc):
    """Cached SPMD dispatcher: the jax.jit closure from
    bass2jax.run_bass_via_pjrt is rebuilt per call there (forcing a retrace,
    executable-cache miss and NEFF reload every dispatch); building it once
    keeps the executable loaded so a dispatch is just transfer + execute."""
    import jax
    from jax.experimental.shard_map import shard_map
    from jax.sharding import Mesh, PartitionSpec
    import concourse.mybir as mybir
    from concourse import bass2jax

    bass2jax.install_neuronx_cc_hook()
    assert nc.dbg_addr is None

    pid_name = nc.partition_id_tensor.name if nc.partition_id_tensor else None
    in_names, out_names, out_avals, zero_shapes = [], [], [], []
    for alloc in nc.m.functions[0].allocations:
        if not isinstance(alloc, mybir.MemoryLocationSet):
            continue
        name = alloc.memorylocations[0].name
        if alloc.kind == "ExternalInput":
            if name != pid_name:
                in_names.append(name)
        elif alloc.kind == "ExternalOutput":
            shape = tuple(alloc.tensor_shape)
            dtype = mybir.dt.np(alloc.dtype)
            out_names.append(name)
            out_avals.append(jax.core.ShapedArray(shape, dtype))
            zero_shapes.append(((NCORES * shape[0],) + shape[1:], dtype))
    n_params = len(in_names)
    all_names = list(in_names) + list(out_names)
    if pid_name is not None:
        all_names.append(pid_name)
    all_names = tuple(all_names)
    donate = tuple(range(n_params, n_params + len(out_names)))

    def _body(*args):
        operands = list(args)
        if pid_name is not None:
            operands.append(bass2jax.partition_id_tensor())
        outs = bass2jax._bass_exec_p.bind(
            *operands, out_avals=tuple(out_avals), in_names=all_names,
            out_names=tuple(out_names), lowering_input_output_aliases=(),
            sim_require_finite=True, sim_require_nnan=True, nc=nc)
        return tuple(outs)

    devices = jax.devices()[:NCORES]
    mesh = Mesh(np.asarray(devices), ("core",))
    in_specs = (PartitionSpec("core"),) * (n_params + len(out_names))
    out_specs = (PartitionSpec("core"),) * len(out_names)
    sharded = jax.jit(shard_map(_body, mesh=mesh, in_specs=in_specs,
                                out_specs=out_specs, check_rep=False),
                      donate_argnums=donate, keep_unused=True)
    oi = out_names.index("out")

    def dispatch(maps):
        concat = [np.concatenate([np.asarray(maps[c][n]) for c in range(NCORES)],
                                 axis=0) for n in in_names]
        zeros = [np.zeros(s, d) for s, d in zero_shapes]
        outs = sharded(*concat, *zeros)
        return np.asarray(outs[oi]).reshape(NCORES, LIGc)

    return dispatch


def _prep_inputs(x, h, t, edges, t_bond, batch_ligand, time_emb_table,
                 W_in, gcl_We1, gcl_Wn1, gcl_We2, gcl_Watt, gcl_Wn2,
                 W_oe, W_f):
    """Host-side sharding of dynamic inputs. Returns list of in_maps."""
    row = np.asarray(edges[0])
    col = np.asarray(edges[1])
    assert np.array_equal(row, np.repeat(np.arange(N), KNN)), "row structure"
    assert np.array_equal(np.asarray(batch_ligand), np.arange(N) // K), "batch structure"
    assert np.all(col // K == row // K), "edges cross ligands"

    # edge time-bond indices (host index arithmetic)
    sbi = row * (K - 1) + col - (row // K) * K - (row < col).astype(row.dtype)
    tbe = np.asarray(t_bond)[sbi].astype(np.int16)        # [E]
    tn = np.asarray(t).astype(np.int16)                   # [N]
    # distances (host)
    xx = np.asarray(x)
    cdiff = xx[row] - xx[col]
    radial = (cdiff ** 2).sum(1)
    dist = np.clip(np.sqrt(radial), 0.0, 4.0).astype(np.float32)
    dhi = dist.astype(bf16)
    dlo = (dist - dhi.astype(np.float32)).astype(bf16)
    col_loc = (col % K).astype(np.float32)

    hh = np.asarray(h)
    maps = []
    for ci in range(NCORES):
        n0 = ci * NLc
        e0 = ci * NEc
        tbe_c = tbe[e0:e0 + NEc]
        idxw0 = np.ascontiguousarray(tbe_c[:16384].reshape(1024, 16).T)
        idxw1 = np.ascontiguousarray(tbe_c[16384:].reshape(1024, 16).T)
        tw = np.ascontiguousarray(tn[n0:n0 + NLc].reshape(256, 16).T)
        idx_all = np.concatenate([idxw0, idxw1, tw], axis=1)  # [16, 2304]
        m = dict(
            hfm=np.ascontiguousarray(hh[n0:n0 + NLc].T).astype(bf16),
            idxs=np.ascontiguousarray(idx_all),
            colv=np.ascontiguousarray(col_loc[e0:e0 + NEc].reshape(1, NEc)).astype(bf16),
            dist2=np.ascontiguousarray(
                np.stack([dhi[e0:e0 + NEc], dlo[e0:e0 + NEc]])),
        )
        maps.append(m)
    return maps


def kernel(x, h, t, edges, t_bond, batch_ligand, num_atoms_per_ligand,
           num_ligands, time_emb_table, W_in, b_in, gcl_We1, gcl_be1, gcl_g1,
           gcl_bt1, gcl_We2, gcl_be2, gcl_Watt, gcl_batt, gcl_Wn1, gcl_bn1,
           gcl_g2, gcl_bt2, gcl_Wn2, gcl_bn2, W_oe, b_oe, W_f, b_f):
    # all biases zero / gains one in this model family; verify then fold away
    for z in (b_in, gcl_be1, gcl_bt1, gcl_be2, gcl_batt, gcl_bn1, gcl_bt2,
              gcl_bn2, b_oe, b_f):
        assert np.abs(np.asarray(z)).max() == 0.0, "nonzero bias unsupported"
    for o in (gcl_g1, gcl_g2):
        assert np.abs(np.asarray(o) - 1.0).max() == 0.0, "non-unit LN gain"
    assert int(num_atoms_per_ligand) == K and int(num_ligands) == N_LIG

    wts = (W_in, gcl_We1, gcl_Wn1, gcl_We2, gcl_Watt, gcl_Wn2, W_oe, W_f,
           time_emb_table)
    import hashlib
    hsh = hashlib.sha1()
    for w in wts:
        hsh.update(np.ascontiguousarray(np.asarray(w, np.float32)).tobytes())
    key = hsh.hexdigest()
    if _COMPILED.get("key") != key:
        _COMPILED["prog"] = _build_program(*wts)
        _COMPILED["dispatch"] = _make_dispatch(_COMPILED["prog"])
        _COMPILED["key"] = key

    maps = _prep_inputs(x, h, t, edges, t_bond, batch_ligand, time_emb_table,
                        W_in, gcl_We1, gcl_Wn1, gcl_We2, gcl_Watt, gcl_Wn2,
                        W_oe, W_f)
    out8 = _COMPILED["dispatch"](maps)
    return np.concatenate(list(out8)).astype(np.float32)
